# revision 8
# baseline (speedup 1.0000x reference)
"""AdditiveAttention on 8 TRN2 NeuronCores.

Math: out = softmax_k(mask(sum_h w_v[h] * tanh(qp[b,q,h] + kp[b,k,h]))) @ values
with qp = queries @ W_q^T, kp = keys @ W_k^T, mask from valid_lens (B,).

Key idea: tanh(q+k) is approximated by a harmonic sine series
    tanh(u) ~= sum_{r=1..R} b_r sin(r*w0*u)
which FACTORIZES via angle addition:
    sin(r*w0*(q+k)) = sin(r*w0*q)cos(r*w0*k) + cos(r*w0*q)sin(r*w0*k)
so the (B,Q,K,H) tanh tensor never materializes: the per-element work is
O((Q+K)*H*R) sin/cos harmonics (ScalarE base + VectorE recurrences) and the
(q,k) coupling is TensorE matmuls with contraction over (h, r, trig).

Harmonics: base sin/cos(w0*x) from ScalarE Sin (args kept inside [-pi,pi] by
choice of w0); odd r via step-2 Chebyshev recurrence S[r+2]=2cos(2th)S[r]-S[r-2]
(VectorE, fp16, 2x mode); even r=2m via doubling sin2m = s_m*c_m (stored scaled
by 1/2^a) and cos2m = 1 - 2*4^a*s_m^2 (ScalarE affine). All scale compensations
and w_v/b_r folding happen in per-partition ACT scale columns computed on host.

Masking: keys are truncated/padded to KP (multiple of 128) >= max(valid_lens);
a rank-1 matmul row adds -60000 to padded score columns, so exp underflows to
exactly 0 like the reference's -1e6 mask.

Sharding: core c handles batch c//2, query rows (c%2)*256..+256.

w0 and the fit coefficients b_r are computed from the actual inputs at call
time (host-side range analysis + least squares), then baked into the program.
"""

import math
from contextlib import ExitStack

import numpy as np

import concourse.bass as bass
import concourse.mybir as mybir
import concourse.tile as tile
from concourse import bacc
from concourse.bass_utils import run_bass_kernel_spmd
from concourse.masks import make_identity

B, Q, K, D, H, V = 4, 512, 512, 256, 256, 256
NCORES = 8
NQ = (B * Q) // NCORES          # 256 query rows per core
R = 12                          # harmonics
NEGM = -60000.0                 # mask add (fits fp16; exp -> exactly 0 in fp32)
FP32 = mybir.dt.float32
FP16 = mybir.dt.float16
AX = mybir.AxisListType
ALU = mybir.AluOpType
ACTF = mybir.ActivationFunctionType

# stored sin_r = sin_r / 2^{A_EXP[r]} from the doubling scheme
A_EXP = {1: 0}
for _r in range(3, R + 1, 2):
    A_EXP[_r] = 0
for _r in range(2, R + 1, 2):
    A_EXP[_r] = A_EXP[_r // 2] + 1
EVENS = list(range(2, R + 1, 2))
ODDS = [r for r in range(3, R + 1, 2)]


def fit_series(qp, kp, vls):
    """Range analysis + least-squares harmonic fit. qp/kp: [b][h, *]."""
    umax, xmax = 0.0, 0.0
    for b in range(B):
        kv = kp[b][:, : vls[b]]
        umax = max(umax, (qp[b].max(1) + kv.max(1)).max(),
                   -(qp[b].min(1) + kv.min(1)).min())
        xmax = max(xmax, np.abs(qp[b]).max(), np.abs(kv).max())
    P = max(2.0 * (umax + 0.15), 4.0 * xmax + 0.08)
    w0 = 2.0 * np.pi / P
    u = np.linspace(-(umax + 0.05), umax + 0.05, 4001)
    A = np.stack([np.sin(r * w0 * u) for r in range(1, R + 1)], 1)
    wgt = np.exp(-(u ** 2) / (2 * 2.5 ** 2)) + 1e-3
    sw = np.sqrt(wgt)[:, None]
    bco, *_ = np.linalg.lstsq(A * sw, np.tanh(u) * sw[:, 0], rcond=None)
    return float(w0), bco.astype(np.float64)


def build_nc(w0, bco, KP):
    """Build the SPMD Bass program. KP = padded key length (multiple of 128)."""
    NK = KP // 128
    QW = 2 * NQ                  # combined q-part width (2 h-chunks)
    CW = QW + 2 * KP             # combined tile width: [hc0 q | hc1 q | hc0 k | hc1 k]
    NCOLA = 2 * R                # per-partition cols: A[hc, r]
    NCOLB = 2 * len(EVENS)       # B[hc, even r]

    nc = bacc.Bacc()
    qT = nc.declare_dram_parameter("qT", [D, NQ], FP32, isOutput=False)
    kT = nc.declare_dram_parameter("kT", [D, KP], FP32, isOutput=False)
    wqT = nc.declare_dram_parameter("wqT", [D, H], FP32, isOutput=False)
    wkT = nc.declare_dram_parameter("wkT", [D, H], FP32, isOutput=False)
    vals = nc.declare_dram_parameter("vals", [KP, V], FP16, isOutput=False)
    mrow = nc.declare_dram_parameter("mrow", [1, KP], FP16, isOutput=False)
    cols = nc.declare_dram_parameter("cols", [128, NCOLA + NCOLB], FP32, isOutput=False)
    out_d = nc.declare_dram_parameter("out", [NQ, V], FP32, isOutput=True)

    def colA(t, hc, r):
        return t[:, hc * R + (r - 1): hc * R + r]

    def colB(t, hc, r):
        j = NCOLA + hc * len(EVENS) + EVENS.index(r)
        return t[:, j: j + 1]

    with TileCtx(nc) as (tc, ctx):
        inp = ctx.enter_context(tc.tile_pool(name="inp", bufs=1))
        harm = ctx.enter_context(tc.tile_pool(name="harm", bufs=1))
        qbp = ctx.enter_context(tc.tile_pool(name="qb", bufs=1))
        tmp_pool = ctx.enter_context(tc.tile_pool(name="tmp", bufs=3))
        sm = ctx.enter_context(tc.tile_pool(name="sm", bufs=1))
        ps_big = ctx.enter_context(tc.tile_pool(name="psA", bufs=1, space="PSUM"))
        ps_sc = ctx.enter_context(tc.tile_pool(name="psS", bufs=1, space="PSUM"))
        ps_pt = ctx.enter_context(tc.tile_pool(name="psT", bufs=2, space="PSUM"))

        # ---- input DMAs ----
        qT_sb = [inp.tile([128, NQ], FP32, tag=f"qT{i}", name=f"qT{i}") for i in range(2)]
        kT_sb = [inp.tile([128, KP], FP32, tag=f"kT{i}", name=f"kT{i}") for i in range(2)]
        wq_sb = [inp.tile([128, H], FP32, tag=f"wq{i}", name=f"wq{i}") for i in range(2)]
        wk_sb = [inp.tile([128, H], FP32, tag=f"wk{i}", name=f"wk{i}") for i in range(2)]
        v_sb = [inp.tile([128, V], FP16, tag=f"v{i}", name=f"v{i}") for i in range(NK)]
        mrow_sb = inp.tile([1, KP], FP16, tag="mrow", name="mrow")
        cols_sb = inp.tile([128, NCOLA + NCOLB], FP32, tag="cols", name="cols")
        for i in range(2):
            nc.sync.dma_start(out=qT_sb[i], in_=qT[128 * i: 128 * (i + 1), :])
            nc.sync.dma_start(out=kT_sb[i], in_=kT[128 * i: 128 * (i + 1), :])
            nc.sync.dma_start(out=wq_sb[i], in_=wqT[128 * i: 128 * (i + 1), :])
            nc.sync.dma_start(out=wk_sb[i], in_=wkT[128 * i: 128 * (i + 1), :])
        for i in range(NK):
            nc.sync.dma_start(out=v_sb[i], in_=vals[128 * i: 128 * (i + 1), :])
        nc.sync.dma_start(out=mrow_sb, in_=mrow[:, :])
        nc.sync.dma_start(out=cols_sb, in_=cols[:, :])

        ident = inp.tile([128, 128], FP16, tag="ident", name="ident")
        make_identity(nc, ident)
        ones_r = inp.tile([1, 128], FP16, tag="ones", name="ones")
        nc.gpsimd.memset(ones_r, 1.0)
        hpi = inp.tile([128, 1], FP32, tag="hpi", name="hpi")
        nc.gpsimd.memset(hpi, math.pi / 2)

        # ---- projections (fp32): qp/kp [h=128 x hc, *] in PSUM ----
        qp_ps, kp_ps = [], []
        for hc in range(2):
            pq = ps_big.tile([128, NQ], FP32, tag=f"bigA{hc}", name=f"qp{hc}")
            pk = ps_big.tile([128, KP], FP32, tag=f"bigK{hc}", name=f"kp{hc}")
            for dc in range(2):
                nc.tensor.matmul(pq, wq_sb[dc][:, 128 * hc: 128 * (hc + 1)],
                                 qT_sb[dc], start=(dc == 0), stop=(dc == 1))
            for dc in range(2):
                nc.tensor.matmul(pk, wk_sb[dc][:, 128 * hc: 128 * (hc + 1)],
                                 kT_sb[dc], start=(dc == 0), stop=(dc == 1))
            qp_ps.append(pq)
            kp_ps.append(pk)

        # ---- base harmonics r=1 (ScalarE Sin; args within [-pi, pi]) ----
        s = {r: harm.tile([128, CW], FP16, tag=f"s{r}", name=f"s{r}") for r in range(1, R + 1)}
        c = {r: harm.tile([128, CW], FP16, tag=f"c{r}", name=f"c{r}") for r in range(1, R + 1)}
        sq = {m: harm.tile([128, CW], FP16, tag=f"sq{m}", name=f"sq{m}") for m in range(1, R // 2 + 1)}
        m2 = harm.tile([128, CW], FP16, tag="m2", name="m2")
        HPI = math.pi / 2
        for hc in range(2):
            qsl = slice(hc * NQ, (hc + 1) * NQ)
            ksl = slice(QW + hc * KP, QW + (hc + 1) * KP)
            nc.scalar.activation(s[1][:, qsl], qp_ps[hc], ACTF.Sin, scale=w0)
            nc.scalar.activation(c[1][:, qsl], qp_ps[hc], ACTF.Sin, scale=w0, bias=hpi)
            nc.scalar.activation(s[1][:, ksl], kp_ps[hc], ACTF.Sin, scale=w0)
            nc.scalar.activation(c[1][:, ksl], kp_ps[hc], ACTF.Sin, scale=w0, bias=hpi)

        # sq1 = s1^2 (DVE); cos2 and 2cos(2th) via ScalarE affine
        nc.vector.tensor_mul(sq[1], s[1], s[1])
        nc.scalar.activation(c[2], sq[1], ACTF.Copy, scale=-2.0, bias=1.0)
        nc.scalar.activation(m2, sq[1], ACTF.Copy, scale=-4.0, bias=2.0)

        # ---- odd chain: S[r+2] = m2*S[r] - S[r-2]  (S[-1] = -S[1], C[-1] = C[1])
        t0 = tmp_pool.tile([128, CW], FP16, tag="tmp", name="tmp")
        nc.vector.tensor_mul(t0, m2, s[1])
        nc.vector.tensor_add(s[3], t0, s[1])
        t1 = tmp_pool.tile([128, CW], FP16, tag="tmp", name="tmp")
        nc.vector.tensor_mul(t1, m2, c[1])
        nc.vector.tensor_sub(c[3], t1, c[1])
        for r in range(5, R + 1, 2):
            ta = tmp_pool.tile([128, CW], FP16, tag="tmp", name="tmp")
            nc.vector.tensor_mul(ta, m2, s[r - 2])
            nc.vector.tensor_sub(s[r], ta, s[r - 4])
            tb = tmp_pool.tile([128, CW], FP16, tag="tmp", name="tmp")
            nc.vector.tensor_mul(tb, m2, c[r - 2])
            nc.vector.tensor_sub(c[r], tb, c[r - 4])

        # ---- even doubling: s[2m] = s[m]*c[m] (scaled); c[2m] = 1-2*4^a*s[m]^2
        for r in EVENS:
            m = r // 2
            nc.vector.tensor_mul(s[r], s[m], c[m])
            if r > 2:
                nc.vector.tensor_mul(sq[m], s[m], s[m])
                nc.scalar.activation(c[r], sq[m], ACTF.Copy,
                                     scale=-2.0 * (4.0 ** A_EXP[m]), bias=1.0)

        # ---- q-side b-scaled stationaries (ScalarE Copy, per-partition cols)
        Sqb = {r: qbp.tile([128, QW], FP16, tag=f"Sb{r}", name=f"Sb{r}") for r in range(1, R + 1)}
        Cqb = {r: qbp.tile([128, QW], FP16, tag=f"Cb{r}", name=f"Cb{r}") for r in range(1, R + 1)}
        for r in range(1, R + 1):
            for hc in range(2):
                qsl = slice(hc * NQ, (hc + 1) * NQ)
                nc.scalar.activation(Sqb[r][:, qsl], s[r][:, qsl], ACTF.Copy,
                                     scale=colA(cols_sb, hc, r))
                if r % 2 == 0:
                    nc.scalar.activation(Cqb[r][:, qsl], sq[r // 2][:, qsl],
                                         ACTF.Identity,
                                         scale=colB(cols_sb, hc, r),
                                         bias=colA(cols_sb, hc, r))
                else:
                    nc.scalar.activation(Cqb[r][:, qsl], c[r][:, qsl], ACTF.Copy,
                                         scale=colA(cols_sb, hc, r))

        # ---- scores: PSUM accumulation of 4R+1 matmuls per q-tile ----
        sc_ps = []
        for qt in range(2):
            sc = ps_sc.tile([128, KP], FP32, tag=f"sc{qt}", name=f"sc{qt}")
            sc_ps.append(sc)
            first = True
            for r in range(1, R + 1):
                for hc in range(2):
                    qsl = slice(hc * NQ + qt * 128, hc * NQ + (qt + 1) * 128)
                    ksl = slice(QW + hc * KP, QW + (hc + 1) * KP)
                    nc.tensor.matmul(sc, Sqb[r][:, qsl], c[r][:, ksl],
                                     start=first, stop=False)
                    first = False
                    nc.tensor.matmul(sc, Cqb[r][:, qsl], s[r][:, ksl],
                                     start=False, stop=False)
            nc.tensor.matmul(sc, ones_r, mrow_sb, start=False, stop=True)

        # ---- softmax + AV per q-tile ----
        for qt in range(2):
            sc = sc_ps[qt]
            negmax = sm.tile([128, 1], FP32, tag=f"nm{qt}", name=f"nm{qt}")
            nc.vector.reduce_max(negmax, sc, axis=AX.X, negate=True)
            p_sb = sm.tile([128, KP], FP16, tag=f"p{qt}", name=f"p{qt}")
            ssum = sm.tile([128, 1], FP32, tag=f"ss{qt}", name=f"ss{qt}")
            nc.scalar.activation(p_sb, sc, ACTF.Exp, bias=negmax, accum_out=ssum)
            rs = sm.tile([128, 1], FP32, tag=f"rs{qt}", name=f"rs{qt}")
            nc.vector.reciprocal(rs, ssum)

            av = ps_big.tile([128, V], FP32, tag=f"bigA{qt}", name=f"av{qt}")
            for kc in range(NK):
                ptp = ps_pt.tile([128, 128], FP16, tag="pt", name="pt")
                nc.tensor.transpose(ptp, p_sb[:, 128 * kc: 128 * (kc + 1)], ident)
                pts = tmp_pool.tile([128, 128], FP16, tag="pts", name="pts")
                nc.vector.tensor_copy(pts, ptp)
                nc.tensor.matmul(av, pts, v_sb[kc],
                                 start=(kc == 0), stop=(kc == NK - 1))
            o_sb = sm.tile([128, V], FP32, tag=f"o{qt}", name=f"o{qt}")
            nc.scalar.activation(o_sb, av, ACTF.Copy, scale=rs)
            nc.sync.dma_start(out=out_d[128 * qt: 128 * (qt + 1), :], in_=o_sb)

    nc.compile()
    return nc


class TileCtx:
    """TileContext + ExitStack in one `with`."""

    def __init__(self, nc):
        self.nc = nc

    def __enter__(self):
        self.ctx = ExitStack()
        self.tc = self.ctx.enter_context(tile.TileContext(self.nc))
        return self.tc, self.ctx

    def __exit__(self, *exc):
        return self.ctx.__exit__(*exc)


def prepare(inputs):
    """Host prep: shards, fit, per-core input maps."""
    queries = np.ascontiguousarray(np.asarray(inputs["queries"], np.float32))
    keys = np.ascontiguousarray(np.asarray(inputs["keys"], np.float32))
    values = np.ascontiguousarray(np.asarray(inputs["values"], np.float32))
    vls = np.asarray(inputs["valid_lens"]).astype(np.int64)
    Wq = np.asarray(inputs["W_q"], np.float32)
    Wk = np.asarray(inputs["W_k"], np.float32)
    wv = np.asarray(inputs["w_v"], np.float32)

    qp = [(Wq @ queries[b].T).astype(np.float32) for b in range(B)]   # [h, q]
    kp = [(Wk @ keys[b].T).astype(np.float32) for b in range(B)]      # [h, k]
    w0, bco = fit_series(qp, kp, vls)
    KP = 128 * max(1, int(math.ceil(vls.max() / 128.0)))

    # per-partition scale columns (same for every core)
    ncolb = len(EVENS)
    cols = np.zeros((128, 2 * R + 2 * ncolb), np.float32)
    for hc in range(2):
        wvh = wv[128 * hc: 128 * (hc + 1)]
        for r in range(1, R + 1):
            cols[:, hc * R + (r - 1)] = wvh * bco[r - 1] * (2.0 ** A_EXP[r])
        for j, r in enumerate(EVENS):
            cols[:, 2 * R + hc * ncolb + j] = (
                -2.0 * (4.0 ** A_EXP[r // 2]) * wvh * bco[r - 1] * (2.0 ** A_EXP[r]))

    in_maps = []
    for core in range(NCORES):
        b, qlo = core // 2, (core % 2) * NQ
        n = int(vls[b])
        kTm = np.zeros((D, KP), np.float32)
        kTm[:, :n] = keys[b, :n].T
        vm = np.zeros((KP, V), np.float16)
        vm[:n] = values[b, :n].astype(np.float16)
        mr = np.where(np.arange(KP) < n, 0.0, NEGM).astype(np.float16)[None, :]
        in_maps.append({
            "qT": np.ascontiguousarray(queries[b, qlo: qlo + NQ].T),
            "kT": kTm,
            "wqT": np.ascontiguousarray(Wq.T),
            "wkT": np.ascontiguousarray(Wk.T),
            "vals": vm,
            "mrow": mr,
            "cols": cols,
        })
    return w0, bco, KP, in_maps


def kernel(**inputs):
    w0, bco, KP, in_maps = prepare(inputs)
    nc = build_nc(w0, bco, KP)
    res = run_bass_kernel_spmd(nc, in_maps, core_ids=list(range(NCORES)))
    out = np.zeros((B, Q, V), np.float32)
    for core in range(NCORES):
        b, qlo = core // 2, (core % 2) * NQ
        out[b, qlo: qlo + NQ] = res.results[core]["out"]
    return out


# revision 15
# speedup vs baseline: 1.3166x; 1.3166x over previous
"""AdditiveAttention on 8 TRN2 NeuronCores.

Math: out = softmax_k(mask(sum_h w_v[h] * tanh(qp[b,q,h] + kp[b,k,h]))) @ values
with qp = queries @ W_q^T, kp = keys @ W_k^T, mask from valid_lens (B,).

Key idea: tanh(q+k) is approximated by a harmonic sine series
    tanh(u) ~= sum_{r=1..R} b_r sin(r*w0*u)
which FACTORIZES via angle addition:
    sin(r*w0*(q+k)) = sin(r*w0*q)cos(r*w0*k) + cos(r*w0*q)sin(r*w0*k)
so the (B,Q,K,H) tanh tensor never materializes: the per-element work is
O((Q+K)*H*R) sin/cos harmonics (ScalarE base + VectorE recurrences) and the
(q,k) coupling is TensorE matmuls with contraction over (h, r, trig).

Harmonics: base sin/cos(w0*x) from ScalarE Sin (args kept inside [-pi,pi] by
choice of w0); odd r via step-2 Chebyshev recurrence S[r+2]=2cos(2th)S[r]-S[r-2]
(VectorE, fp16, 2x mode); even r=2m via doubling sin2m = s_m*c_m (stored scaled
by 1/2^a) and cos2m = 1 - 2*4^a*s_m^2 (ScalarE affine). All scale compensations
and w_v/b_r folding happen in per-partition ACT scale columns computed on host.

Masking: keys are truncated/padded to KP (multiple of 128) >= max(valid_lens);
a rank-1 matmul row adds -60000 to padded score columns, so exp underflows to
exactly 0 like the reference's -1e6 mask.

Sharding: core c handles batch c//2, query rows (c%2)*256..+256.

w0 and the fit coefficients b_r are computed from the actual inputs at call
time (host-side range analysis + least squares), then baked into the program.
"""

import math
from contextlib import ExitStack

import numpy as np

import concourse.bass as bass
import concourse.mybir as mybir
import concourse.tile as tile
from concourse import bacc
from concourse.bass_utils import run_bass_kernel_spmd
from concourse.masks import make_identity

B, Q, K, D, H, V = 4, 512, 512, 256, 256, 256
NCORES = 8
NQ = (B * Q) // NCORES          # 256 query rows per core
R = 10                          # harmonics
NEGM = -60000.0                 # mask add (fits fp16; exp -> exactly 0 in fp32)
FP32 = mybir.dt.float32
FP16 = mybir.dt.float16
AX = mybir.AxisListType
ALU = mybir.AluOpType
ACTF = mybir.ActivationFunctionType

# stored sin_r = sin_r / 2^{A_EXP[r]} from the doubling scheme
A_EXP = {1: 0}
for _r in range(3, R + 1, 2):
    A_EXP[_r] = 0
for _r in range(2, R + 1, 2):
    A_EXP[_r] = A_EXP[_r // 2] + 1
EVENS = list(range(2, R + 1, 2))
ODDS = [r for r in range(3, R + 1, 2)]


def fit_series(qp, kp, vls):
    """Range analysis + least-squares harmonic fit. qp/kp: [b][h, *]."""
    umax, xmax = 0.0, 0.0
    for b in range(B):
        kv = kp[b][:, : vls[b]]
        umax = max(umax, (qp[b].max(1) + kv.max(1)).max(),
                   -(qp[b].min(1) + kv.min(1)).min())
        xmax = max(xmax, np.abs(qp[b]).max(), np.abs(kv).max())
    P = max(2.0 * (umax + 0.15), 4.0 * xmax + 0.08)
    w0 = 2.0 * np.pi / P
    u = np.linspace(-(umax + 0.05), umax + 0.05, 4001)
    A = np.stack([np.sin(r * w0 * u) for r in range(1, R + 1)], 1)
    wgt = np.exp(-(u ** 2) / (2 * 2.5 ** 2)) + 1e-3
    sw = np.sqrt(wgt)[:, None]
    bco, *_ = np.linalg.lstsq(A * sw, np.tanh(u) * sw[:, 0], rcond=None)
    return float(w0), bco.astype(np.float64)


def build_nc(w0, bco, KP):
    """Build the SPMD Bass program. KP = padded key length (multiple of 128)."""
    NK = KP // 128
    QW = 2 * NQ                  # combined q-part width (2 h-chunks)
    CW = QW + 2 * KP             # combined tile width: [hc0 q | hc1 q | hc0 k | hc1 k]
    NCOLA = 2 * R                # per-partition cols: A[hc, r]
    NCOLB = 2 * len(EVENS)       # B[hc, even r]

    nc = bacc.Bacc()
    qT = nc.declare_dram_parameter("qT", [D, NQ], FP16, isOutput=False)
    kT = nc.declare_dram_parameter("kT", [D, KP], FP16, isOutput=False)
    wqT = nc.declare_dram_parameter("wqT", [D, H], FP16, isOutput=False)
    wkT = nc.declare_dram_parameter("wkT", [D, H], FP16, isOutput=False)
    vals = nc.declare_dram_parameter("vals", [KP, V], FP16, isOutput=False)
    mrow = nc.declare_dram_parameter("mrow", [1, KP], FP16, isOutput=False)
    cols = nc.declare_dram_parameter("cols", [128, NCOLA + NCOLB], FP32, isOutput=False)
    out_d = nc.declare_dram_parameter("out", [NQ, V], FP32, isOutput=True)

    def colA(t, hc, r):
        return t[:, hc * R + (r - 1): hc * R + r]

    def colB(t, hc, r):
        j = NCOLA + hc * len(EVENS) + EVENS.index(r)
        return t[:, j: j + 1]

    with TileCtx(nc) as (tc, ctx):
        inp = ctx.enter_context(tc.tile_pool(name="inp", bufs=1))
        harm = ctx.enter_context(tc.tile_pool(name="harm", bufs=1))
        qbp = ctx.enter_context(tc.tile_pool(name="qb", bufs=1))
        tmp_pool = ctx.enter_context(tc.tile_pool(name="tmp", bufs=3))
        sm = ctx.enter_context(tc.tile_pool(name="sm", bufs=1))
        ps_big = ctx.enter_context(tc.tile_pool(name="psA", bufs=1, space="PSUM"))
        ps_sc = ctx.enter_context(tc.tile_pool(name="psS", bufs=1, space="PSUM"))
        ps_pt = ctx.enter_context(tc.tile_pool(name="psT", bufs=2, space="PSUM"))

        # ---- input DMAs ----
        qT_sb = [inp.tile([128, NQ], FP16, tag=f"qT{i}", name=f"qT{i}") for i in range(2)]
        kT_sb = [inp.tile([128, KP], FP16, tag=f"kT{i}", name=f"kT{i}") for i in range(2)]
        wq_sb = [inp.tile([128, H], FP16, tag=f"wq{i}", name=f"wq{i}") for i in range(2)]
        wk_sb = [inp.tile([128, H], FP16, tag=f"wk{i}", name=f"wk{i}") for i in range(2)]
        v_sb = [inp.tile([128, V], FP16, tag=f"v{i}", name=f"v{i}") for i in range(NK)]
        mrow_sb = inp.tile([1, KP], FP16, tag="mrow", name="mrow")
        cols_sb = inp.tile([128, NCOLA + NCOLB], FP32, tag="cols", name="cols")
        for i in range(2):
            nc.sync.dma_start(out=qT_sb[i], in_=qT[128 * i: 128 * (i + 1), :])
            nc.sync.dma_start(out=kT_sb[i], in_=kT[128 * i: 128 * (i + 1), :])
            nc.sync.dma_start(out=wq_sb[i], in_=wqT[128 * i: 128 * (i + 1), :])
            nc.sync.dma_start(out=wk_sb[i], in_=wkT[128 * i: 128 * (i + 1), :])
        for i in range(NK):
            nc.sync.dma_start(out=v_sb[i], in_=vals[128 * i: 128 * (i + 1), :])
        nc.sync.dma_start(out=mrow_sb, in_=mrow[:, :])
        nc.sync.dma_start(out=cols_sb, in_=cols[:, :])

        ident = inp.tile([128, 128], FP16, tag="ident", name="ident")
        make_identity(nc, ident)
        ones_r = inp.tile([1, 128], FP16, tag="ones", name="ones")
        nc.gpsimd.memset(ones_r, 1.0)
        hpi = inp.tile([128, 1], FP32, tag="hpi", name="hpi")
        nc.gpsimd.memset(hpi, math.pi / 2)

        # ---- projections (fp32): qp/kp [h=128 x hc, *] in PSUM ----
        qp_ps, kp_ps = [], []
        for hc in range(2):
            pq = ps_big.tile([128, NQ], FP32, tag=f"bigA{hc}", name=f"qp{hc}")
            pk = ps_big.tile([128, KP], FP32, tag=f"bigK{hc}", name=f"kp{hc}")
            for dc in range(2):
                nc.tensor.matmul(pq, wq_sb[dc][:, 128 * hc: 128 * (hc + 1)],
                                 qT_sb[dc], start=(dc == 0), stop=(dc == 1))
            for dc in range(2):
                nc.tensor.matmul(pk, wk_sb[dc][:, 128 * hc: 128 * (hc + 1)],
                                 kT_sb[dc], start=(dc == 0), stop=(dc == 1))
            qp_ps.append(pq)
            kp_ps.append(pk)

        # ---- base harmonics r=1 (ScalarE Sin; args within [-pi, pi]) ----
        s = {r: harm.tile([128, CW], FP16, tag=f"s{r}", name=f"s{r}") for r in range(1, R + 1)}
        c = {r: harm.tile([128, CW], FP16, tag=f"c{r}", name=f"c{r}") for r in range(1, R + 1)}
        sq = {m: harm.tile([128, CW], FP16, tag=f"sq{m}", name=f"sq{m}") for m in range(1, R // 2 + 1)}
        m2 = harm.tile([128, CW], FP16, tag="m2", name="m2")
        HPI = math.pi / 2
        for hc in range(2):
            qsl = slice(hc * NQ, (hc + 1) * NQ)
            ksl = slice(QW + hc * KP, QW + (hc + 1) * KP)
            nc.scalar.activation(s[1][:, qsl], qp_ps[hc], ACTF.Sin, scale=w0)
            nc.scalar.activation(c[1][:, qsl], qp_ps[hc], ACTF.Sin, scale=w0, bias=hpi)
            nc.scalar.activation(s[1][:, ksl], kp_ps[hc], ACTF.Sin, scale=w0)
            nc.scalar.activation(c[1][:, ksl], kp_ps[hc], ACTF.Sin, scale=w0, bias=hpi)

        # ---- harmonics r=2..R on DVE, r-ascending for pipelining.
        # odd chain: S[r+2] = m2*S[r] - S[r-2] (S[1]=-S[-1], C[-1]=C[1]);
        # even doubling: s[2m] = s[m]*c[m] (scaled 1/2^a); c[2m] affine of s[m]^2
        # (fused 2-op tensor_scalar on DVE).
        nc.vector.tensor_mul(sq[1], s[1], s[1])
        nc.vector.tensor_mul(s[2], s[1], c[1])
        nc.vector.tensor_scalar(c[2], sq[1], -2.0, 1.0, ALU.mult, ALU.add)
        nc.vector.tensor_scalar(m2, sq[1], -4.0, 2.0, ALU.mult, ALU.add)
        t0 = tmp_pool.tile([128, CW], FP16, tag="tmp", name="tmp")
        nc.vector.tensor_mul(t0, m2, s[1])
        nc.vector.tensor_add(s[3], t0, s[1])
        t1 = tmp_pool.tile([128, CW], FP16, tag="tmp", name="tmp")
        nc.vector.tensor_mul(t1, m2, c[1])
        nc.vector.tensor_sub(c[3], t1, c[1])
        for r in range(4, R + 1):
            if r % 2 == 0:
                m = r // 2
                nc.vector.tensor_mul(sq[m], s[m], s[m])
                nc.vector.tensor_mul(s[r], s[m], c[m])
                nc.vector.tensor_scalar(c[r], sq[m], -2.0 * (4.0 ** A_EXP[m]), 1.0,
                                        ALU.mult, ALU.add)
            else:
                ta = tmp_pool.tile([128, CW], FP16, tag="tmp", name="tmp")
                nc.vector.tensor_mul(ta, m2, s[r - 2])
                nc.vector.tensor_sub(s[r], ta, s[r - 4])
                tb = tmp_pool.tile([128, CW], FP16, tag="tmp", name="tmp")
                nc.vector.tensor_mul(tb, m2, c[r - 2])
                nc.vector.tensor_sub(c[r], tb, c[r - 4])

        # ---- q-side b-scaled stationaries (ScalarE Copy, per-partition cols)
        Sqb = {r: qbp.tile([128, QW], FP16, tag=f"Sb{r}", name=f"Sb{r}") for r in range(1, R + 1)}
        Cqb = {r: qbp.tile([128, QW], FP16, tag=f"Cb{r}", name=f"Cb{r}") for r in range(1, R + 1)}
        for r in range(1, R + 1):
            for hc in range(2):
                qsl = slice(hc * NQ, (hc + 1) * NQ)
                nc.scalar.activation(Sqb[r][:, qsl], s[r][:, qsl], ACTF.Copy,
                                     scale=colA(cols_sb, hc, r))
                if r % 2 == 0:
                    nc.scalar.activation(Cqb[r][:, qsl], sq[r // 2][:, qsl],
                                         ACTF.Identity,
                                         scale=colB(cols_sb, hc, r),
                                         bias=colA(cols_sb, hc, r))
                else:
                    nc.scalar.activation(Cqb[r][:, qsl], c[r][:, qsl], ACTF.Copy,
                                         scale=colA(cols_sb, hc, r))

        # ---- scores: PSUM accumulation of 4R+1 matmuls per q-tile ----
        sc_ps = []
        for qt in range(2):
            sc = ps_sc.tile([128, KP], FP32, tag=f"sc{qt}", name=f"sc{qt}")
            sc_ps.append(sc)
            # mask row first: PE starts (and warms) right after input DMAs
            nc.tensor.matmul(sc, ones_r, mrow_sb, start=True, stop=False)
            for r in range(1, R + 1):
                for hc in range(2):
                    qsl = slice(hc * NQ + qt * 128, hc * NQ + (qt + 1) * 128)
                    ksl = slice(QW + hc * KP, QW + (hc + 1) * KP)
                    nc.tensor.matmul(sc, Sqb[r][:, qsl], c[r][:, ksl],
                                     start=False, stop=False)
                    nc.tensor.matmul(sc, Cqb[r][:, qsl], s[r][:, ksl],
                                     start=False, stop=(r == R and hc == 1))

        # ---- softmax + AV per q-tile ----
        for qt in range(2):
            sc = sc_ps[qt]
            negmax = sm.tile([128, 1], FP32, tag=f"nm{qt}", name=f"nm{qt}")
            nc.vector.reduce_max(negmax, sc, axis=AX.X, negate=True)
            p_sb = sm.tile([128, KP], FP16, tag=f"p{qt}", name=f"p{qt}")
            ssum = sm.tile([128, 1], FP32, tag=f"ss{qt}", name=f"ss{qt}")
            nc.scalar.activation(p_sb, sc, ACTF.Exp, bias=negmax, accum_out=ssum)
            rs = sm.tile([128, 1], FP32, tag=f"rs{qt}", name=f"rs{qt}")
            nc.vector.reciprocal(rs, ssum)

            av = ps_big.tile([128, V], FP32, tag=f"bigA{qt}", name=f"av{qt}")
            for kc in range(NK):
                ptp = ps_pt.tile([128, 128], FP16, tag="pt", name="pt")
                nc.tensor.transpose(ptp, p_sb[:, 128 * kc: 128 * (kc + 1)], ident)
                pts = tmp_pool.tile([128, 128], FP16, tag="pts", name="pts")
                nc.vector.tensor_copy(pts, ptp)
                nc.tensor.matmul(av, pts, v_sb[kc],
                                 start=(kc == 0), stop=(kc == NK - 1))
            o_sb = sm.tile([128, V], FP32, tag=f"o{qt}", name=f"o{qt}")
            nc.scalar.activation(o_sb, av, ACTF.Copy, scale=rs)
            nc.sync.dma_start(out=out_d[128 * qt: 128 * (qt + 1), :], in_=o_sb)

    nc.compile()
    return nc


class TileCtx:
    """TileContext + ExitStack in one `with`."""

    def __init__(self, nc):
        self.nc = nc

    def __enter__(self):
        self.ctx = ExitStack()
        self.tc = self.ctx.enter_context(tile.TileContext(self.nc))
        return self.tc, self.ctx

    def __exit__(self, *exc):
        return self.ctx.__exit__(*exc)


def prepare(inputs):
    """Host prep: shards, fit, per-core input maps."""
    queries = np.ascontiguousarray(np.asarray(inputs["queries"], np.float32))
    keys = np.ascontiguousarray(np.asarray(inputs["keys"], np.float32))
    values = np.ascontiguousarray(np.asarray(inputs["values"], np.float32))
    vls = np.asarray(inputs["valid_lens"]).astype(np.int64)
    Wq = np.asarray(inputs["W_q"], np.float32)
    Wk = np.asarray(inputs["W_k"], np.float32)
    wv = np.asarray(inputs["w_v"], np.float32)

    # device projections run on fp16-rounded inputs; match that for ranges
    q16 = queries.astype(np.float16).astype(np.float32)
    k16 = keys.astype(np.float16).astype(np.float32)
    Wq16 = Wq.astype(np.float16).astype(np.float32)
    Wk16 = Wk.astype(np.float16).astype(np.float32)
    qp = [(Wq16 @ q16[b].T).astype(np.float32) for b in range(B)]   # [h, q]
    kp = [(Wk16 @ k16[b].T).astype(np.float32) for b in range(B)]   # [h, k]
    w0, bco = fit_series(qp, kp, vls)
    KP = 128 * max(1, int(math.ceil(vls.max() / 128.0)))

    # per-partition scale columns (same for every core)
    ncolb = len(EVENS)
    cols = np.zeros((128, 2 * R + 2 * ncolb), np.float32)
    for hc in range(2):
        wvh = wv[128 * hc: 128 * (hc + 1)]
        for r in range(1, R + 1):
            cols[:, hc * R + (r - 1)] = wvh * bco[r - 1] * (2.0 ** A_EXP[r])
        for j, r in enumerate(EVENS):
            cols[:, 2 * R + hc * ncolb + j] = (
                -2.0 * (4.0 ** A_EXP[r // 2]) * wvh * bco[r - 1] * (2.0 ** A_EXP[r]))

    in_maps = []
    for core in range(NCORES):
        b, qlo = core // 2, (core % 2) * NQ
        n = int(vls[b])
        kTm = np.zeros((D, KP), np.float16)
        kTm[:, :n] = keys[b, :n].T.astype(np.float16)
        vm = np.zeros((KP, V), np.float16)
        vm[:n] = values[b, :n].astype(np.float16)
        mr = np.where(np.arange(KP) < n, 0.0, NEGM).astype(np.float16)[None, :]
        in_maps.append({
            "qT": np.ascontiguousarray(queries[b, qlo: qlo + NQ].T.astype(np.float16)),
            "kT": kTm,
            "wqT": np.ascontiguousarray(Wq.T.astype(np.float16)),
            "wkT": np.ascontiguousarray(Wk.T.astype(np.float16)),
            "vals": vm,
            "mrow": mr,
            "cols": cols,
        })
    return w0, bco, KP, in_maps


def kernel(**inputs):
    w0, bco, KP, in_maps = prepare(inputs)
    nc = build_nc(w0, bco, KP)
    res = run_bass_kernel_spmd(nc, in_maps, core_ids=list(range(NCORES)))
    out = np.zeros((B, Q, V), np.float32)
    for core in range(NCORES):
        b, qlo = core // 2, (core % 2) * NQ
        out[b, qlo: qlo + NQ] = res.results[core]["out"]
    return out
